# revision 6
# baseline (speedup 1.0000x reference)
"""Trainium2 Bass kernel for nn_Decoder (teacher-forced 2-layer LSTM decoder
with 32000-vocab projection, summed per-step masked CE + greedy argmax).

Sharding (8 NeuronCores, SPMD, one program + per-core data):
- LSTM recurrence: sharded over the hidden dim (each core owns 128 of 1024
  h-dims, both layers). 49 pipelined ticks; layer-1 runs one step behind
  layer-0 so ONE AllGather per tick exchanges both fresh h chunks.
  Recurrent matmuls are 3-pass bf16-split (w_h*h_h + w_h*h_l + w_l*h_h) for
  ~fp32 gate accuracy. The layer-0 input term X @ w_ih0^T is precomputed
  batched over all 48 (padded) steps.
- Vocab projection: sharded over vocab (4000 real rows/core, padded 4096).
  Single-pass bf16 matmuls -> f32 logits per 128-row tile; per-tile top-8
  (values + indices) and exp-sum via one activation pass with accumulate.
  One stats AllGather, then every core redundantly merges global logsumexp
  and global top-3 candidates.
- Exact rescore: each core rescoes 3 of the 24 row-tiles (selected via
  per-core host index tensors + indirect DMA gather/scatter): top-3
  candidates + target get exact fp32 dots (gathered W_out rows x
  reconstructed f32 h1 rows), making the argmax and the CE target term
  immune to the bf16 projection error. Loss partials AllReduce-summed.
"""
import sys
import numpy as np

if "/opt/trn_rl_repo" not in sys.path:
    sys.path.insert(0, "/opt/trn_rl_repo")

import ml_dtypes  # noqa: E402
import concourse.bass as bass  # noqa: E402
import concourse.bacc as bacc  # noqa: E402
import concourse.tile as tile  # noqa: E402
import concourse.mybir as mybir  # noqa: E402
from concourse import bass_utils  # noqa: E402

F32 = mybir.dt.float32
BF16 = mybir.dt.bfloat16
U32 = mybir.dt.uint32
I32 = mybir.dt.int32
AX = mybir.AxisListType
OP = mybir.AluOpType
ACT = mybir.ActivationFunctionType

NCORES = 8
B = 64            # batch
TS = 47           # real decode steps (MAX_LEN-1)
T = 48            # padded steps (step 47 is a dummy)
TB = T * B        # 3072 rows; row r = t*64 + b
NT = TB // 128    # 24 row-tiles of 128 rows
H = 1024
KC = 8            # 128-wide contraction chunks over H
VREAL = 4000      # real vocab rows per core
VPC = 4096        # padded vocab rows per core
VOCAB = 32000
NEG_BIG = -1.0e30
TPC = NT // NCORES  # row-tiles rescored per core (3)

BF = ml_dtypes.bfloat16

_CACHE = {}


def build_nc():
    if "nc" in _CACHE:
        return _CACHE["nc"]
    nc = bacc.Bacc("TRN2", target_bir_lowering=False, debug=False,
                   num_devices=NCORES)

    # ---------------- I/O ----------------
    xTh_in = nc.dram_tensor("xTh_in", [KC, 128, TB], BF16, kind="ExternalInput").ap()
    xTl_in = nc.dram_tensor("xTl_in", [KC, 128, TB], BF16, kind="ExternalInput").ap()
    wih0h_in = nc.dram_tensor("wih0h_in", [32, 128, 128], BF16, kind="ExternalInput").ap()
    wih0l_in = nc.dram_tensor("wih0l_in", [32, 128, 128], BF16, kind="ExternalInput").ap()
    wrech_in = nc.dram_tensor("wrech_in", [96, 128, 128], BF16, kind="ExternalInput").ap()
    wrecl_in = nc.dram_tensor("wrecl_in", [96, 128, 128], BF16, kind="ExternalInput").ap()
    wproj_in = nc.dram_tensor("wproj_in", [KC, 128, VPC], BF16, kind="ExternalInput").ap()
    wout_in = nc.dram_tensor("wout_in", [VOCAB, H], F32, kind="ExternalInput").ap()
    h0init_in = nc.dram_tensor("h0init_in", [128, KC, 2, 64], BF16, kind="ExternalInput").ap()
    h1init_in = nc.dram_tensor("h1init_in", [128, KC, 2, 64], BF16, kind="ExternalInput").ap()
    h1chunk_in = nc.dram_tensor("h1chunk_in", [128, 64], F32, kind="ExternalInput").ap()
    cinit_in = nc.dram_tensor("cinit_in", [128, 2, 64], F32, kind="ExternalInput").ap()
    vbase_in = nc.dram_tensor("vbase_in", [128, 1], F32, kind="ExternalInput").ap()
    tgtf_in = nc.dram_tensor("tgtf_in", [128, NT], F32, kind="ExternalInput").ap()
    lossw_in = nc.dram_tensor("lossw_in", [128, NT], F32, kind="ExternalInput").ap()
    # per-core rescore selectors
    rowsel_in = nc.dram_tensor("rowsel_in", [128, TPC], F32, kind="ExternalInput").ap()
    tmask_in = nc.dram_tensor("tmask_in", [128, TPC, NT], F32, kind="ExternalInput").ap()

    loss_out = nc.dram_tensor("loss_out", [1, 1], F32, kind="ExternalOutput").ap()
    result_out = nc.dram_tensor("result_out", [T, B], I32, kind="ExternalOutput").ap()

    # internal DRAM
    g0x_d = nc.dram_tensor("g0x_d", [4, 128, TB], F32).ap()
    h1rh_d = nc.dram_tensor("h1rh_d", [TB, H], BF16).ap()
    h1rl_d = nc.dram_tensor("h1rl_d", [TB, H], BF16).ap()

    with tile.TileContext(nc) as tc:
        with (
            tc.tile_pool(name="persist", bufs=1) as per,
            tc.tile_pool(name="dram", bufs=2, space="DRAM") as dram,
        ):
            # ---- persistent SBUF (~98KB/partition) ----
            wrech = per.tile([128, 96, 128], BF16, tag="wrech")
            nc.sync.dma_start(wrech[:], wrech_in.rearrange("n p m -> p n m"))
            wrecl = per.tile([128, 96, 128], BF16, tag="wrecl")
            nc.sync.dma_start(wrecl[:], wrecl_in.rearrange("n p m -> p n m"))
            h1Th = per.tile([128, KC, TB], BF16, tag="h1Th")

            tgtf = per.tile([128, NT], F32, tag="tgtf")
            nc.sync.dma_start(tgtf[:], tgtf_in)
            lossw = per.tile([128, NT], F32, tag="lossw")
            nc.sync.dma_start(lossw[:], lossw_in)
            vbase = per.tile([128, 1], F32, tag="vbase")
            nc.sync.dma_start(vbase[:], vbase_in)
            rowsel = per.tile([128, TPC], F32, tag="rowsel")
            nc.sync.dma_start(rowsel[:], rowsel_in)
            tmask = per.tile([128, TPC, NT], F32, tag="tmask")
            nc.sync.dma_start(tmask[:], tmask_in)

            top8v = per.tile([128, NT, 8], F32, tag="top8v")
            top8i = per.tile([128, NT, 8], U32, tag="top8i")
            sumexp = per.tile([128, NT], F32, tag="sumexp")
            lse = per.tile([128, NT], F32, tag="lse")
            candi = per.tile([128, 3, NT], F32, tag="candi")

            # ============ Phase 1: g0x = (X @ w_ih0^T)^T, batched ============
            with (
                tc.tile_pool(name="pre_sb", bufs=2) as psb,
                tc.tile_pool(name="pre_fold", bufs=3) as pfb,
                tc.tile_pool(name="pre_ps", bufs=2, space="PSUM") as pps,
                tc.tile_pool(name="pre_w", bufs=1) as pw,
            ):
                wih0h = pw.tile([128, 32, 128], BF16, tag="wih0h")
                nc.sync.dma_start(wih0h[:], wih0h_in.rearrange("n p m -> p n m"))
                wih0l = pw.tile([128, 32, 128], BF16, tag="wih0l")
                nc.sync.dma_start(wih0l[:], wih0l_in.rearrange("n p m -> p n m"))

                for ch in range(TB // 512):
                    cs = ch * 512
                    xh = psb.tile([128, KC, 512], BF16, tag="xh")
                    nc.sync.dma_start(
                        xh[:], xTh_in[:, :, cs:cs + 512].rearrange("k p c -> p k c"))
                    xl = psb.tile([128, KC, 512], BF16, tag="xl")
                    nc.sync.dma_start(
                        xl[:], xTl_in[:, :, cs:cs + 512].rearrange("k p c -> p k c"))
                    for gt in range(4):
                        ps = pps.tile([128, 2, 512], F32, tag="ps")
                        for kc in range(KC):
                            idx = gt * 8 + kc
                            nc.tensor.matmul(ps[:, 0, :], wih0h[:, idx, :],
                                             xh[:, kc, :], start=(kc == 0),
                                             stop=False, skip_group_check=True)
                            nc.tensor.matmul(ps[:, 1, :], wih0h[:, idx, :],
                                             xl[:, kc, :], start=(kc == 0),
                                             stop=False, skip_group_check=True)
                            nc.tensor.matmul(ps[:, 0, :], wih0l[:, idx, :],
                                             xh[:, kc, :], start=False,
                                             stop=(kc == KC - 1),
                                             skip_group_check=True)
                        fold = pfb.tile([128, 512], F32, tag="fold")
                        nc.any.tensor_copy(fold[:], ps[:, 1, :])
                        gsum = pfb.tile([128, 512], F32, tag="gsum")
                        nc.vector.tensor_tensor(gsum[:], ps[:, 0, :], fold[:],
                                                OP.add)
                        nc.sync.dma_start(g0x_d[gt, :, cs:cs + 512], gsum[:])

            # wproj loads after phase-1 scratch is freed (used in phase 3)
            with tc.tile_pool(name="w2", bufs=1) as w2:
                wproj = w2.tile([128, KC, VPC], BF16, tag="wproj")
                nc.sync.dma_start(wproj[:], wproj_in.rearrange("k p v -> p k v"))

                # ============ Phase 2: recurrence ticks ============
                with (
                    tc.tile_pool(name="rec_sb", bufs=1) as rsb,
                    tc.tile_pool(name="rec_pf", bufs=2) as rpf,
                    tc.tile_pool(name="rec_st", bufs=1) as rst,
                    tc.tile_pool(name="rec_ps", bufs=2, space="PSUM") as rps,
                    tc.tile_pool(name="prj_sb", bufs=1) as jsb,
                    tc.tile_pool(name="prj_sm", bufs=2) as jsm,
                    tc.tile_pool(name="prj_ps", bufs=1, space="PSUM") as jps,
                ):
                    def do_projection(tau):
                        logits = jsb.tile([128, VPC], F32, tag="logits")
                        for half in range(2):
                            pj = jps.tile([128, 4, 512], F32, tag="pp")
                            for kc in range(KC):
                                for vq in range(4):
                                    vs = half * 2048 + vq * 512
                                    nc.tensor.matmul(
                                        pj[:, vq, :],
                                        h1Th[:, kc, tau * 128:(tau + 1) * 128],
                                        wproj[:, kc, vs:vs + 512],
                                        start=(kc == 0), stop=(kc == KC - 1),
                                        skip_group_check=True)
                            nc.any.tensor_copy(
                                logits[:, half * 2048:(half + 1) * 2048],
                                pj[:].rearrange("p v n -> p (v n)"))
                        nc.vector.memset(logits[:, VREAL:VPC], NEG_BIG)
                        nc.vector.max(top8v[:, tau, :], logits[:])
                        nc.vector.max_index(top8i[:, tau, :],
                                            top8v[:, tau, :], logits[:])
                        negm = jsm.tile([128, 1], F32, tag="negm")
                        nc.vector.tensor_scalar_mul(negm[:],
                                                    top8v[:, tau, 0:1], -1.0)
                        s12 = jsm.tile([128, 2], F32, tag="s12")
                        for hh in range(2):
                            escr = jsb.tile([128, VPC // 2], BF16, tag="escr")
                            nc.scalar.activation(
                                escr[:], logits[:, hh * 2048:(hh + 1) * 2048],
                                ACT.Exp, bias=negm[:], scale=1.0,
                                accum_out=s12[:, hh:hh + 1])
                        nc.vector.tensor_tensor(sumexp[:, tau:tau + 1],
                                                s12[:, 0:1], s12[:, 1:2],
                                                OP.add)

                    h0buf = rst.tile([128, KC, 2, 64], BF16, tag="h0buf")
                    nc.sync.dma_start(h0buf[:], h0init_in)
                    h1buf = rst.tile([128, KC, 2, 64], BF16, tag="h1buf")
                    nc.sync.dma_start(h1buf[:], h1init_in)
                    c01 = rst.tile([128, 2, 64], F32, tag="c01")
                    nc.sync.dma_start(c01[:], cinit_in)
                    h1chunk0 = rst.tile([128, 64], F32, tag="h1chunk0")
                    nc.sync.dma_start(h1chunk0[:], h1chunk_in)
                    h1lrot = rst.tile([128, KC, 2, 64], BF16, tag="h1lrot")

                    for t in range(T + 1):
                        l0 = t <= T - 1
                        l1 = t >= 1
                        lo_s = 0 if l0 else 4
                        hi_s = 8 if l1 else 4
                        lyr = slice(0 if l0 else 1, 2 if l1 else 1)
                        ps = rps.tile([128, 8, 2, 64], F32, tag="tick")

                        if l0:
                            for s in range(4):
                                for kc in range(KC):
                                    idx = s * 8 + kc
                                    nc.tensor.matmul(
                                        ps[:, s, :, :], wrech[:, idx, :],
                                        h0buf[:, kc, :, :], start=(kc == 0),
                                        stop=False, skip_group_check=True)
                                    nc.tensor.matmul(
                                        ps[:, s, 0, :], wrecl[:, idx, :],
                                        h0buf[:, kc, 0, :], start=False,
                                        stop=(kc == KC - 1),
                                        skip_group_check=True)
                        if l1:
                            for s in range(4):
                                for kc in range(16):
                                    idx = 32 + s * 16 + kc
                                    src = h0buf if kc < 8 else h1buf
                                    kk = kc % 8
                                    nc.tensor.matmul(
                                        ps[:, 4 + s, :, :], wrech[:, idx, :],
                                        src[:, kk, :, :], start=(kc == 0),
                                        stop=False, skip_group_check=True)
                                    nc.tensor.matmul(
                                        ps[:, 4 + s, 0, :], wrecl[:, idx, :],
                                        src[:, kk, 0, :], start=False,
                                        stop=(kc == 15), skip_group_check=True)

                        # fold the wh*hl half into the main half, add x-term
                        fold = rsb.tile([128, 8, 64], F32, tag="tfold")
                        nc.any.tensor_copy(fold[:, lo_s:hi_s, :],
                                           ps[:, lo_s:hi_s, 1, :])
                        gsum = rsb.tile([128, 8, 64], F32, tag="tgsum")
                        nc.vector.tensor_tensor(gsum[:, lo_s:hi_s, :],
                                                ps[:, lo_s:hi_s, 0, :],
                                                fold[:, lo_s:hi_s, :], OP.add)
                        if l0:
                            g0xb = rpf.tile([128, 4, 64], F32, tag="g0xb")
                            nc.sync.dma_start(
                                g0xb[:], g0x_d[:, :, t * 64:(t + 1) * 64]
                                .rearrange("g p b -> p g b"))
                            nc.vector.tensor_tensor(gsum[:, 0:4, :],
                                                    gsum[:, 0:4, :], g0xb[:],
                                                    OP.add)
                        # gates (slices i,f,o,g): sigmoid via tanh(x/2)
                        sig = rsb.tile([128, 2, 3, 64], F32, tag="sig")
                        gth = rsb.tile([128, 2, 64], F32, tag="gth")
                        if l0:
                            nc.scalar.activation(sig[:, 0, :, :], gsum[:, 0:3, :],
                                                 ACT.Tanh, bias=0.0, scale=0.5)
                            nc.scalar.activation(gth[:, 0, :], gsum[:, 3, :],
                                                 ACT.Tanh)
                        if l1:
                            nc.scalar.activation(sig[:, 1, :, :], gsum[:, 4:7, :],
                                                 ACT.Tanh, bias=0.0, scale=0.5)
                            nc.scalar.activation(gth[:, 1, :], gsum[:, 7, :],
                                                 ACT.Tanh)
                        nc.vector.tensor_scalar(sig[:, lyr, :, :],
                                                sig[:, lyr, :, :],
                                                0.5, 0.5, OP.mult, OP.add)
                        t1 = rsb.tile([128, 2, 64], F32, tag="t1")
                        nc.vector.tensor_tensor(t1[:, lyr, :], sig[:, lyr, 0, :],
                                                gth[:, lyr, :], OP.mult)
                        t2 = rsb.tile([128, 2, 64], F32, tag="t2")
                        nc.vector.tensor_tensor(t2[:, lyr, :], sig[:, lyr, 1, :],
                                                c01[:, lyr, :], OP.mult)
                        nc.vector.tensor_tensor(c01[:, lyr, :], t1[:, lyr, :],
                                                t2[:, lyr, :], OP.add)
                        ct = rsb.tile([128, 2, 64], F32, tag="ct")
                        nc.scalar.activation(ct[:, lyr, :], c01[:, lyr, :],
                                             ACT.Tanh)
                        hloc = rsb.tile([128, 2, 64], F32, tag="hloc")
                        nc.vector.tensor_tensor(hloc[:, lyr, :],
                                                sig[:, lyr, 2, :],
                                                ct[:, lyr, :], OP.mult)
                        if t == 0:
                            nc.vector.tensor_copy(hloc[:, 1, :], h1chunk0[:])
                        if t == T:
                            nc.vector.tensor_copy(hloc[:, 0, :], hloc[:, 1, :])

                        # exchange
                        contrib = dram.tile([128, 128], F32, tag="contrib")
                        gathered = dram.tile([1024, 128], F32, tag="gathered")
                        nc.sync.dma_start(contrib[:], hloc[:])
                        nc.gpsimd.collective_compute(
                            "AllGather", OP.bypass,
                            replica_groups=[list(range(NCORES))],
                            ins=[contrib.opt()], outs=[gathered.opt()])
                        gat_r = gathered.rearrange("(k p) c -> p k c", p=128)
                        if t < T:
                            h0g = rsb.tile([128, KC, 64], F32, tag="h0g")
                            nc.sync.dma_start(h0g[:], gat_r[:, :, 0:64])
                            nc.vector.tensor_copy(h0buf[:, :, 0, :], h0g[:])
                            nc.vector.tensor_tensor(h0buf[:, :, 1, :], h0g[:],
                                                    h0buf[:, :, 0, :],
                                                    OP.subtract)
                        h1g = rsb.tile([128, KC, 64], F32, tag="h1g")
                        nc.sync.dma_start(h1g[:], gat_r[:, :, 64:128])
                        s_h1 = t - 1  # step of the gathered h1
                        if s_h1 >= 0:
                            cp = s_h1 * 64
                            nc.vector.tensor_copy(h1Th[:, :, cp:cp + 64], h1g[:])
                            nc.vector.tensor_copy(h1buf[:, :, 0, :],
                                                  h1Th[:, :, cp:cp + 64])
                            nc.vector.tensor_tensor(h1buf[:, :, 1, :], h1g[:],
                                                    h1buf[:, :, 0, :],
                                                    OP.subtract)
                            nc.vector.tensor_copy(h1lrot[:, :, s_h1 % 2, :],
                                                  h1buf[:, :, 1, :])
                        # transpose finished 128-row tile (steps 2tau, 2tau+1)
                        if s_h1 >= 1 and s_h1 % 2 == 1:
                            tau = s_h1 // 2
                            sth = rsb.tile([128, H], BF16, tag="sth")
                            stl = rsb.tile([128, H], BF16, tag="stl")
                            for kc in range(KC):
                                nc.sync.dma_start(
                                    sth[:, kc * 128:(kc + 1) * 128],
                                    h1Th[:, kc, 2 * tau * 64:2 * tau * 64 + 128],
                                    transpose=True)
                                nc.sync.dma_start(
                                    stl[:, kc * 128:(kc + 1) * 128],
                                    h1lrot[:, kc, :, :], transpose=True)
                            nc.sync.dma_start(
                                h1rh_d[tau * 128:(tau + 1) * 128, :], sth[:])
                            nc.sync.dma_start(
                                h1rl_d[tau * 128:(tau + 1) * 128, :], stl[:])
                            do_projection(tau)

                # ============ Phase 4: stats exchange + combine ============
                NQ = 2 * NT + 2 * NT * 8  # 432
                with tc.tile_pool(name="cmb_sb", bufs=1) as csb:
                    cont = csb.tile([128, NQ], F32, tag="scont")
                    nc.vector.tensor_copy(cont[:, 0:NT],
                                          top8v[:, :, 0:1].rearrange("p t o -> p (t o)"))
                    nc.vector.tensor_copy(cont[:, NT:2 * NT], sumexp[:])
                    nc.vector.tensor_copy(
                        cont[:, 2 * NT:2 * NT + 192],
                        top8v[:].rearrange("p t k -> p (t k)"))
                    gidxf = csb.tile([128, NT, 8], F32, tag="gidxf")
                    nc.vector.tensor_copy(gidxf[:], top8i[:])
                    nc.vector.tensor_scalar(gidxf[:], gidxf[:], vbase[:], None,
                                            OP.add)
                    nc.vector.tensor_copy(
                        cont[:, 2 * NT + 192:NQ],
                        gidxf[:].rearrange("p t k -> p (t k)"))
                    scont_d = dram.tile([128, NQ], F32, tag="scont_d")
                    sgath_d = dram.tile([128 * NCORES, NQ], F32, tag="sgath_d")
                    nc.sync.dma_start(scont_d[:], cont[:])
                    nc.gpsimd.collective_compute(
                        "AllGather", OP.bypass,
                        replica_groups=[list(range(NCORES))],
                        ins=[scont_d.opt()], outs=[sgath_d.opt()])
                    gst = csb.tile([128, NCORES, NQ], F32, tag="gst")
                    nc.sync.dma_start(
                        gst[:], sgath_d.rearrange("(c p) q -> p c q", p=128))

                    # global logsumexp per row
                    maxv_v = gst[:, :, 0:NT].rearrange("p c t -> p t c")
                    sume_v = gst[:, :, NT:2 * NT].rearrange("p c t -> p t c")
                    gm = csb.tile([128, NT], F32, tag="gm")
                    nc.vector.tensor_reduce(gm[:], maxv_v, axis=AX.X, op=OP.max)
                    dsc = csb.tile([128, NT, NCORES], F32, tag="dsc")
                    nc.vector.tensor_tensor(
                        dsc[:], maxv_v,
                        gm[:].unsqueeze(2).broadcast_to((128, NT, NCORES)),
                        OP.subtract)
                    nc.scalar.activation(dsc[:], dsc[:], ACT.Exp)
                    nc.vector.tensor_tensor(dsc[:], dsc[:], sume_v, OP.mult)
                    S = csb.tile([128, NT], F32, tag="S")
                    nc.vector.tensor_reduce(S[:], dsc[:], axis=AX.X, op=OP.add)
                    nc.scalar.activation(S[:], S[:], ACT.Ln)
                    nc.vector.tensor_tensor(lse[:], S[:], gm[:], OP.add)

                    # global top-3 (value order) with first-occurrence index
                    cand_v = gst[:, :, 2 * NT:2 * NT + 192].rearrange(
                        "p c (t k) -> p t c k", t=NT)
                    gidx_v = gst[:, :, 2 * NT + 192:NQ].rearrange(
                        "p c (t k) -> p t c k", t=NT)
                    work = csb.tile([128, NT, NCORES, 8], F32, tag="work")
                    nc.vector.tensor_copy(work[:], cand_v)
                    scr1 = csb.tile([128, NT, NCORES, 8], F32, tag="scr1")
                    scr2 = csb.tile([128, NT, NCORES, 8], F32, tag="scr2")
                    isel = csb.tile([128, NT, NCORES, 8], F32, tag="isel")
                    for r in range(3):
                        vr = csb.tile([128, NT], F32, tag="vr")
                        nc.vector.tensor_reduce(vr[:], work[:], axis=AX.XY,
                                                op=OP.max)
                        nc.vector.tensor_tensor(
                            scr1[:], work[:],
                            vr[:].unsqueeze(2).unsqueeze(3).broadcast_to(
                                (128, NT, NCORES, 8)), OP.is_equal)
                        nc.vector.tensor_scalar(scr2[:], scr1[:], -1e9, 1e9,
                                                OP.mult, OP.add)
                        nc.vector.tensor_tensor(isel[:], gidx_v, scr1[:],
                                                OP.mult)
                        nc.vector.tensor_tensor(isel[:], isel[:], scr2[:],
                                                OP.add)
                        nc.vector.tensor_reduce(candi[:, r, :], isel[:],
                                                axis=AX.XY, op=OP.min)
                        if r < 2:
                            nc.vector.tensor_scalar(scr1[:], scr1[:], 2e30,
                                                    None, OP.mult)
                            nc.vector.tensor_tensor(work[:], work[:], scr1[:],
                                                    OP.subtract)

            # ============ Phase 5: rescore + loss + tokens ============
            with (
                tc.tile_pool(name="rsc_sb", bufs=2) as ssb,
                tc.tile_pool(name="rsc_st", bufs=1) as sst,
                tc.tile_pool(name="rsc_ps", bufs=1, space="PSUM") as sps,
            ):
                # select this core's 3 row-tiles from the [*, NT] stats
                def msel(src_nt, name):
                    """src [128, NT] -> [128, TPC] via per-core one-hot mask."""
                    scr = ssb.tile([128, TPC, NT], F32, tag="mscr")
                    nc.vector.tensor_tensor(
                        scr[:], src_nt.unsqueeze(1).broadcast_to(
                            (128, TPC, NT)), tmask[:], OP.mult)
                    out = sst.tile([128, TPC], F32, tag=name)
                    nc.vector.tensor_reduce(out[:], scr[:], axis=AX.X, op=OP.add)
                    return out

                tgt_s = msel(tgtf[:], "tgt_s")
                lw_s = msel(lossw[:], "lw_s")
                lse_s = msel(lse[:], "lse_s")
                cand_s = [msel(candi[:, r, :], f"cand_s{r}") for r in range(3)]

                rowu = sst.tile([128, TPC], U32, tag="rowu")
                nc.vector.tensor_copy(rowu[:], rowsel[:])
                scat = sst.tile([128, TPC], F32, tag="scat")
                nc.vector.tensor_scalar(scat[:], rowsel[:], float(B), None,
                                        OP.add)
                scatu = sst.tile([128, TPC], U32, tag="scatu")
                nc.vector.tensor_copy(scatu[:], scat[:])

                dots = sst.tile([128, TPC, 4], F32, tag="dots")
                for i in range(TPC):
                    h1h_t = ssb.tile([128, H], BF16, tag="h1h_t")
                    nc.gpsimd.indirect_dma_start(
                        out=h1h_t[:], out_offset=None, in_=h1rh_d,
                        in_offset=bass.IndirectOffsetOnAxis(
                            ap=rowu[:, i:i + 1], axis=0))
                    h1l_t = ssb.tile([128, H], BF16, tag="h1l_t")
                    nc.gpsimd.indirect_dma_start(
                        out=h1l_t[:], out_offset=None, in_=h1rl_d,
                        in_offset=bass.IndirectOffsetOnAxis(
                            ap=rowu[:, i:i + 1], axis=0))
                    h1row = ssb.tile([128, H], F32, tag="h1row")
                    nc.vector.tensor_copy(h1row[:], h1h_t[:])
                    nc.vector.tensor_tensor(h1row[:], h1row[:], h1l_t[:],
                                            OP.add)
                    for ci in range(4):
                        src = cand_s[ci][:, i:i + 1] if ci < 3 \
                            else tgt_s[:, i:i + 1]
                        cu = ssb.tile([128, 1], U32, tag="cu")
                        nc.vector.tensor_copy(cu[:], src)
                        wrow = ssb.tile([128, H], F32, tag="wrow")
                        nc.gpsimd.indirect_dma_start(
                            out=wrow[:], out_offset=None, in_=wout_in,
                            in_offset=bass.IndirectOffsetOnAxis(
                                ap=cu[:, 0:1], axis=0))
                        prod = ssb.tile([128, H], F32, tag="prod")
                        nc.vector.tensor_tensor(prod[:], h1row[:], wrow[:],
                                                OP.mult)
                        nc.vector.tensor_reduce(dots[:, i, ci:ci + 1], prod[:],
                                                axis=AX.X, op=OP.add)

                # winner among the 3 rescored candidates (exact values)
                best = sst.tile([128, TPC], F32, tag="best")
                nc.vector.tensor_copy(best[:], dots[:, :, 0])
                bidx = sst.tile([128, TPC], F32, tag="bidx")
                nc.vector.tensor_copy(bidx[:], cand_s[0][:])
                for ci in (1, 2):
                    m = sst.tile([128, TPC], U32, tag="m")
                    nc.vector.tensor_tensor(m[:], dots[:, :, ci], best[:],
                                            OP.is_gt)
                    nc.vector.copy_predicated(best[:], m[:], dots[:, :, ci])
                    nc.vector.copy_predicated(bidx[:], m[:], cand_s[ci][:])
                toki = sst.tile([128, TPC], I32, tag="toki")
                nc.vector.tensor_copy(toki[:], bidx[:])
                # scatter tokens into result[1+t, b] (flat offset r + 64)
                res_flat = result_out.rearrange("t b -> (t b)").unsqueeze(1)
                for i in range(TPC):
                    nc.gpsimd.indirect_dma_start(
                        out=res_flat, out_offset=bass.IndirectOffsetOnAxis(
                            ap=scatu[:, i:i + 1], axis=0),
                        in_=toki[:, i:i + 1], in_offset=None,
                        bounds_check=TB - 1, oob_is_err=False)
                ones_row = sst.tile([64, 1], I32, tag="ones_row")
                nc.vector.memset(ones_row[:], 1)
                nc.sync.dma_start(res_flat[0:B], ones_row[:])

                # loss = sum over rows of (lse - exact_tgt_dot) * lossw
                ce = sst.tile([128, TPC], F32, tag="ce")
                nc.vector.tensor_tensor(ce[:], lse_s[:], dots[:, :, 3],
                                        OP.subtract)
                nc.vector.tensor_tensor(ce[:], ce[:], lw_s[:], OP.mult)
                part = sst.tile([128, 1], F32, tag="part")
                nc.vector.tensor_reduce(part[:], ce[:], axis=AX.X, op=OP.add)
                ar_in = dram.tile([128, 1], F32, tag="ar_in")
                ar_out = dram.tile([128, 1], F32, tag="ar_out")
                nc.sync.dma_start(ar_in[:], part[:])
                nc.gpsimd.collective_compute(
                    "AllReduce", OP.add,
                    replica_groups=[list(range(NCORES))],
                    ins=[ar_in.opt()], outs=[ar_out.opt()])
                summed = sst.tile([128, 1], F32, tag="summed")
                nc.sync.dma_start(summed[:], ar_out[:])
                onesf = sst.tile([128, 1], F32, tag="onesf")
                nc.vector.memset(onesf[:], 1.0)
                lps = sps.tile([1, 1], F32, tag="lps")
                nc.tensor.matmul(lps[:], onesf[:], summed[:], start=True,
                                 stop=True)
                lsb = sst.tile([1, 1], F32, tag="lsb")
                nc.vector.tensor_copy(lsb[:], lps[:])
                nc.sync.dma_start(loss_out, lsb[:])

    nc.compile()
    _CACHE["nc"] = nc
    return nc


def _split(x):
    h = np.asarray(x, np.float32).astype(BF)
    l = (np.asarray(x, np.float32) - h.astype(np.float32)).astype(BF)
    return h, l


def prep_inputs(output_tensor, hidden_state, cell_state, embedding,
                w_ih, w_hh, W_out, **_unused):
    """Host-side sharding/layout prep. Returns per-core input maps."""
    tok = np.asarray(output_tensor)
    emb = np.asarray(embedding, np.float32).copy()
    emb[0] = 0.0
    w_ih = np.asarray(w_ih, np.float32)
    w_hh = np.asarray(w_hh, np.float32)
    W_out = np.ascontiguousarray(np.asarray(W_out, np.float32))
    h_st = np.asarray(hidden_state, np.float32)
    c_st = np.asarray(cell_state, np.float32)

    tok_in = np.concatenate([tok[:TS].T.reshape(-1),
                             np.zeros(B, np.int64)])  # [TB] t-major, b fast
    # careful: rows r = t*64+b -> in token order tok[t, b]
    tok_in = np.zeros(TB, np.int64)
    tgt = np.zeros(TB, np.int64)
    tgrid = np.asarray(tok)
    for t in range(TS):
        tok_in[t * B:(t + 1) * B] = tgrid[t]
        tgt[t * B:(t + 1) * B] = tgrid[t + 1]
    # step 47 (dummy): tokens 0, tgt 0 (masked)

    X = emb[tok_in]                       # [TB, 1024]
    XT = np.ascontiguousarray(X.T)        # [1024, TB]
    xh, xl = _split(XT)
    xTh = xh.reshape(KC, 128, TB)
    xTl = xl.reshape(KC, 128, TB)

    # per-row loss weights: mask/denom ; tgtf
    mask = (tgt != 0).astype(np.float32)
    mask[TS * B:] = 0.0
    lw = np.zeros(TB, np.float32)
    for t in range(TS):
        s = mask[t * B:(t + 1) * B].sum()
        lw[t * B:(t + 1) * B] = mask[t * B:(t + 1) * B] / max(s, 1.0)
    # SBUF layout [128 partitions, NT]: row r = tau*128 + p
    lw_pt = lw.reshape(NT, 128).T.copy()           # [128, NT]
    tgt_pt = tgt.astype(np.float32).reshape(NT, 128).T.copy()

    # gate-block order in this kernel: i, f, o, g  (PyTorch rows: i,f,g,o)
    GT_ROWS = [0, H, 3 * H, 2 * H]  # start row of i, f, o, g in [4H]

    in_maps = []
    for j in range(NCORES):
        hj = slice(128 * j, 128 * (j + 1))

        def rec_tiles(mats):
            """mats: list of (matrix [4H or G, K], kc-range) stacked tiles."""
            tiles_h = []
            tiles_l = []
            for s, g0 in enumerate(GT_ROWS):
                for (mat, kcn) in mats:
                    rows = mat[g0 + 128 * j: g0 + 128 * (j + 1)]  # [128, K]
                    for kc in range(kcn):
                        blk = rows[:, kc * 128:(kc + 1) * 128].T  # [128k,128m]
                        bh, bl = _split(blk)
                        tiles_h.append(bh)
                        tiles_l.append(bl)
            return np.stack(tiles_h), np.stack(tiles_l)

        wih0h_t, wih0l_t = rec_tiles([(w_ih[0], KC)])
        wrec_l0h, wrec_l0l = rec_tiles([(w_hh[0], KC)])
        wrec_l1h, wrec_l1l = rec_tiles([(w_ih[1], KC), (w_hh[1], KC)])
        wrech_t = np.concatenate([wrec_l0h, wrec_l1h])
        wrecl_t = np.concatenate([wrec_l0l, wrec_l1l])

        Wj = np.zeros((VPC, H), np.float32)
        Wj[:VREAL] = W_out[VREAL * j:VREAL * (j + 1)]
        wpj_h = Wj.T.astype(BF)                      # [1024, 4096] bf16
        wproj_t = wpj_h.reshape(KC, 128, VPC)

        h0T = np.ascontiguousarray(h_st[0].T)        # [1024, 64]
        h1T = np.ascontiguousarray(h_st[1].T)
        def init_pack(hT):
            hh, hl = _split(hT)
            out = np.zeros((128, KC, 2, B), BF)
            for kc in range(KC):
                out[:, kc, 0, :] = hh[kc * 128:(kc + 1) * 128]
                out[:, kc, 1, :] = hl[kc * 128:(kc + 1) * 128]
            return out
        cinit = np.zeros((128, 2, B), np.float32)
        cinit[:, 0, :] = c_st[0].T[hj]
        cinit[:, 1, :] = c_st[1].T[hj]

        rowsel = np.zeros((128, TPC), np.float32)
        tmask_np = np.zeros((128, TPC, NT), np.float32)
        for i in range(TPC):
            tau = TPC * j + i
            rowsel[:, i] = 128 * tau + np.arange(128)
            tmask_np[:, i, tau] = 1.0

        in_maps.append({
            "xTh_in": xTh, "xTl_in": xTl,
            "wih0h_in": wih0h_t, "wih0l_in": wih0l_t,
            "wrech_in": wrech_t, "wrecl_in": wrecl_t,
            "wproj_in": wproj_t, "wout_in": W_out,
            "h0init_in": init_pack(h0T), "h1init_in": init_pack(h1T),
            "h1chunk_in": np.ascontiguousarray(h1T[hj]),
            "cinit_in": cinit,
            "vbase_in": np.full((128, 1), VREAL * j, np.float32),
            "tgtf_in": tgt_pt, "lossw_in": lw_pt,
            "rowsel_in": rowsel, "tmask_in": tmask_np,
        })
    return in_maps


def assemble(results, out_dtype):
    """Per-core results -> (loss [1] f32, result [48,64] out_dtype)."""
    loss = np.asarray(results[0]["loss_out"]).reshape(1).astype(np.float32)
    result = np.ones((TS + 1, B), np.int64)
    for j in range(NCORES):
        res_j = np.asarray(results[j]["result_out"])  # [48, 64]
        for i in range(TPC):
            tau = TPC * j + i
            for half in range(2):
                s = 2 * tau + half          # step index
                if s >= TS:
                    continue
                result[1 + s] = res_j[1 + s]
    return loss, result.astype(out_dtype)


def kernel(**inputs):
    nc = build_nc()
    in_maps = prep_inputs(**inputs)
    r = bass_utils.run_bass_kernel_spmd(
        nc, in_maps, core_ids=list(range(NCORES)))
    out_dtype = np.asarray(inputs["output_tensor"]).dtype
    return assemble(r.results, out_dtype)


if __name__ == "__main__":
    build_nc()
    print("built ok")


# revision 9
# speedup vs baseline: 1.0039x; 1.0039x over previous
"""Trainium2 Bass kernel for nn_Decoder (teacher-forced 2-layer LSTM decoder
with 32000-vocab projection, summed per-step masked CE + greedy argmax).

Sharding (8 NeuronCores, SPMD, one program + per-core data):
- LSTM recurrence: sharded over the hidden dim (each core owns 128 of 1024
  h-dims, both layers). 49 pipelined ticks; layer-1 runs one step behind
  layer-0 so ONE AllGather per tick exchanges both fresh h chunks.
  Recurrent matmuls are 3-pass bf16-split (w_h*h_h + w_h*h_l + w_l*h_h) for
  ~fp32 gate accuracy. The layer-0 input term X @ w_ih0^T is precomputed
  batched over all 48 (padded) steps.
- Vocab projection: sharded over vocab (4000 real rows/core, padded 4096).
  Single-pass bf16 matmuls -> f32 logits per 128-row tile; per-tile top-8
  (values + indices) and exp-sum via one activation pass with accumulate.
  One stats AllGather, then every core redundantly merges global logsumexp
  and global top-3 candidates.
- Exact rescore: each core rescoes 3 of the 24 row-tiles (selected via
  per-core host index tensors + indirect DMA gather/scatter): top-3
  candidates + target get exact fp32 dots (gathered W_out rows x
  reconstructed f32 h1 rows), making the argmax and the CE target term
  immune to the bf16 projection error. Loss partials AllReduce-summed.
"""
import sys
import numpy as np

if "/opt/trn_rl_repo" not in sys.path:
    sys.path.insert(0, "/opt/trn_rl_repo")

import ml_dtypes  # noqa: E402
import concourse.bass as bass  # noqa: E402
import concourse.bacc as bacc  # noqa: E402
import concourse.tile as tile  # noqa: E402
import concourse.mybir as mybir  # noqa: E402
from concourse import bass_utils  # noqa: E402

F32 = mybir.dt.float32
BF16 = mybir.dt.bfloat16
U32 = mybir.dt.uint32
I32 = mybir.dt.int32
AX = mybir.AxisListType
OP = mybir.AluOpType
ACT = mybir.ActivationFunctionType

NCORES = 8
B = 64            # batch
TS = 47           # real decode steps (MAX_LEN-1)
T = 48            # padded steps (step 47 is a dummy)
TB = T * B        # 3072 rows; row r = t*64 + b
NT = TB // 128    # 24 row-tiles of 128 rows
H = 1024
KC = 8            # 128-wide contraction chunks over H
VREAL = 4000      # real vocab rows per core
VPC = 4096        # padded vocab rows per core
VOCAB = 32000
NEG_BIG = -1.0e30
TPC = NT // NCORES  # row-tiles rescored per core (3)

BF = ml_dtypes.bfloat16

_CACHE = {}


def build_nc():
    if "nc" in _CACHE:
        return _CACHE["nc"]
    nc = bacc.Bacc("TRN2", target_bir_lowering=False, debug=False,
                   num_devices=NCORES)

    # ---------------- I/O ----------------
    xTh_in = nc.dram_tensor("xTh_in", [KC, 128, TB], BF16, kind="ExternalInput").ap()
    xTl_in = nc.dram_tensor("xTl_in", [KC, 128, TB], BF16, kind="ExternalInput").ap()
    wih0h_in = nc.dram_tensor("wih0h_in", [32, 128, 128], BF16, kind="ExternalInput").ap()
    wih0l_in = nc.dram_tensor("wih0l_in", [32, 128, 128], BF16, kind="ExternalInput").ap()
    wrech_in = nc.dram_tensor("wrech_in", [96, 128, 128], BF16, kind="ExternalInput").ap()
    wrecl_in = nc.dram_tensor("wrecl_in", [96, 128, 128], BF16, kind="ExternalInput").ap()
    wproj_in = nc.dram_tensor("wproj_in", [KC, 128, VPC], BF16, kind="ExternalInput").ap()
    wout_in = nc.dram_tensor("wout_in", [VOCAB, H], F32, kind="ExternalInput").ap()
    h0init_in = nc.dram_tensor("h0init_in", [128, KC, 2, 64], BF16, kind="ExternalInput").ap()
    h1init_in = nc.dram_tensor("h1init_in", [128, KC, 2, 64], BF16, kind="ExternalInput").ap()
    h1chunk_in = nc.dram_tensor("h1chunk_in", [128, 64], F32, kind="ExternalInput").ap()
    cinit_in = nc.dram_tensor("cinit_in", [128, 2, 64], F32, kind="ExternalInput").ap()
    vbase_in = nc.dram_tensor("vbase_in", [128, 1], F32, kind="ExternalInput").ap()
    tgtf_in = nc.dram_tensor("tgtf_in", [128, NT], F32, kind="ExternalInput").ap()
    lossw_in = nc.dram_tensor("lossw_in", [128, NT], F32, kind="ExternalInput").ap()
    # per-core rescore selectors
    rowsel_in = nc.dram_tensor("rowsel_in", [128, TPC], F32, kind="ExternalInput").ap()
    tmask_in = nc.dram_tensor("tmask_in", [128, TPC, NT], F32, kind="ExternalInput").ap()

    loss_out = nc.dram_tensor("loss_out", [1, 1], F32, kind="ExternalOutput").ap()
    result_out = nc.dram_tensor("result_out", [T, B], I32, kind="ExternalOutput").ap()

    # internal DRAM
    g0x_d = nc.dram_tensor("g0x_d", [4, 128, TB], F32).ap()
    h1rh_d = nc.dram_tensor("h1rh_d", [TB, H], BF16).ap()
    h1rl_d = nc.dram_tensor("h1rl_d", [TB, H], BF16).ap()

    with tile.TileContext(nc) as tc:
        with (
            tc.tile_pool(name="persist", bufs=1) as per,
            tc.tile_pool(name="dram", bufs=2, space="DRAM") as dram,
        ):
            # ---- persistent SBUF (~98KB/partition) ----
            wrech = per.tile([128, 96, 128], BF16, tag="wrech")
            nc.sync.dma_start(wrech[:], wrech_in.rearrange("n p m -> p n m"))
            wrecl = per.tile([128, 96, 128], BF16, tag="wrecl")
            nc.sync.dma_start(wrecl[:], wrecl_in.rearrange("n p m -> p n m"))
            h1Th = per.tile([128, KC, TB], BF16, tag="h1Th")

            tgtf = per.tile([128, NT], F32, tag="tgtf")
            nc.sync.dma_start(tgtf[:], tgtf_in)
            lossw = per.tile([128, NT], F32, tag="lossw")
            nc.sync.dma_start(lossw[:], lossw_in)
            vbase = per.tile([128, 1], F32, tag="vbase")
            nc.sync.dma_start(vbase[:], vbase_in)
            rowsel = per.tile([128, TPC], F32, tag="rowsel")
            nc.sync.dma_start(rowsel[:], rowsel_in)
            tmask = per.tile([128, TPC, NT], F32, tag="tmask")
            nc.sync.dma_start(tmask[:], tmask_in)

            top8v = per.tile([128, NT, 8], F32, tag="top8v")
            top8i = per.tile([128, NT, 8], U32, tag="top8i")
            sumexp = per.tile([128, NT], F32, tag="sumexp")
            lse = per.tile([128, NT], F32, tag="lse")
            candi = per.tile([128, 3, NT], F32, tag="candi")

            # ============ Phase 1: g0x = (X @ w_ih0^T)^T, batched ============
            with (
                tc.tile_pool(name="pre_sb", bufs=2) as psb,
                tc.tile_pool(name="pre_fold", bufs=3) as pfb,
                tc.tile_pool(name="pre_ps", bufs=2, space="PSUM") as pps,
                tc.tile_pool(name="pre_w", bufs=1) as pw,
            ):
                wih0h = pw.tile([128, 32, 128], BF16, tag="wih0h")
                nc.sync.dma_start(wih0h[:], wih0h_in.rearrange("n p m -> p n m"))
                wih0l = pw.tile([128, 32, 128], BF16, tag="wih0l")
                nc.sync.dma_start(wih0l[:], wih0l_in.rearrange("n p m -> p n m"))

                for ch in range(TB // 512):
                    cs = ch * 512
                    xh = psb.tile([128, KC, 512], BF16, tag="xh")
                    nc.sync.dma_start(
                        xh[:], xTh_in[:, :, cs:cs + 512].rearrange("k p c -> p k c"))
                    xl = psb.tile([128, KC, 512], BF16, tag="xl")
                    nc.sync.dma_start(
                        xl[:], xTl_in[:, :, cs:cs + 512].rearrange("k p c -> p k c"))
                    for gt in range(4):
                        ps = pps.tile([128, 2, 512], F32, tag="ps")
                        for kc in range(KC):
                            idx = gt * 8 + kc
                            nc.tensor.matmul(ps[:, 0, :], wih0h[:, idx, :],
                                             xh[:, kc, :], start=(kc == 0),
                                             stop=False, skip_group_check=True)
                            nc.tensor.matmul(ps[:, 1, :], wih0h[:, idx, :],
                                             xl[:, kc, :], start=(kc == 0),
                                             stop=False, skip_group_check=True)
                            nc.tensor.matmul(ps[:, 0, :], wih0l[:, idx, :],
                                             xh[:, kc, :], start=False,
                                             stop=(kc == KC - 1),
                                             skip_group_check=True)
                        fold = pfb.tile([128, 512], F32, tag="fold")
                        nc.any.tensor_copy(fold[:], ps[:, 1, :])
                        gsum = pfb.tile([128, 512], F32, tag="gsum")
                        nc.vector.tensor_tensor(gsum[:], ps[:, 0, :], fold[:],
                                                OP.add)
                        nc.sync.dma_start(g0x_d[gt, :, cs:cs + 512], gsum[:])

            # wproj loads after phase-1 scratch is freed (used in phase 3)
            with tc.tile_pool(name="w2", bufs=1) as w2:
                wproj = w2.tile([128, KC, VPC], BF16, tag="wproj")
                nc.sync.dma_start(wproj[:], wproj_in.rearrange("k p v -> p k v"))

                # ============ Phase 2: recurrence ticks ============
                with (
                    tc.tile_pool(name="rec_sb", bufs=1) as rsb,
                    tc.tile_pool(name="rec_pf", bufs=2) as rpf,
                    tc.tile_pool(name="rec_st", bufs=1) as rst,
                    tc.tile_pool(name="rec_ps", bufs=2, space="PSUM") as rps,
                    tc.tile_pool(name="prj_sb", bufs=1) as jsb,
                    tc.tile_pool(name="prj_sm", bufs=2) as jsm,
                    tc.tile_pool(name="prj_ps", bufs=1, space="PSUM") as jps,
                ):
                    def do_projection(tau):
                        logits = jsb.tile([128, VPC], F32, tag="logits")
                        for half in range(2):
                            pj = jps.tile([128, 4, 512], F32, tag="pp")
                            for kc in range(KC):
                                for vq in range(4):
                                    vs = half * 2048 + vq * 512
                                    nc.tensor.matmul(
                                        pj[:, vq, :],
                                        h1Th[:, kc, tau * 128:(tau + 1) * 128],
                                        wproj[:, kc, vs:vs + 512],
                                        start=(kc == 0), stop=(kc == KC - 1),
                                        skip_group_check=True)
                            nc.any.tensor_copy(
                                logits[:, half * 2048:(half + 1) * 2048],
                                pj[:].rearrange("p v n -> p (v n)"))
                        nc.vector.memset(logits[:, VREAL:VPC], NEG_BIG)
                        nc.vector.max(top8v[:, tau, :], logits[:])
                        nc.vector.max_index(top8i[:, tau, :],
                                            top8v[:, tau, :], logits[:])
                        negm = jsm.tile([128, 1], F32, tag="negm")
                        nc.vector.tensor_scalar_mul(negm[:],
                                                    top8v[:, tau, 0:1], -1.0)
                        s12 = jsm.tile([128, 2], F32, tag="s12")
                        for hh in range(2):
                            escr = jsb.tile([128, VPC // 2], BF16, tag="escr")
                            nc.scalar.activation(
                                escr[:], logits[:, hh * 2048:(hh + 1) * 2048],
                                ACT.Exp, bias=negm[:], scale=1.0,
                                accum_out=s12[:, hh:hh + 1])
                        nc.vector.tensor_tensor(sumexp[:, tau:tau + 1],
                                                s12[:, 0:1], s12[:, 1:2],
                                                OP.add)

                    h0buf = rst.tile([128, KC, 2, 64], BF16, tag="h0buf")
                    nc.sync.dma_start(h0buf[:], h0init_in)
                    h1buf = rst.tile([128, KC, 2, 64], BF16, tag="h1buf")
                    nc.sync.dma_start(h1buf[:], h1init_in)
                    c01 = rst.tile([128, 2, 64], F32, tag="c01")
                    nc.sync.dma_start(c01[:], cinit_in)
                    h1chunk0 = rst.tile([128, 64], F32, tag="h1chunk0")
                    nc.sync.dma_start(h1chunk0[:], h1chunk_in)
                    h1lrot = rst.tile([128, KC, 2, 64], BF16, tag="h1lrot")

                    for t in range(T + 1):
                        l0 = t <= T - 1
                        l1 = t >= 1
                        lo_s = 0 if l0 else 4
                        hi_s = 8 if l1 else 4
                        lyr = slice(0 if l0 else 1, 2 if l1 else 1)
                        ps = rps.tile([128, 8, 2, 64], F32, tag="tick")

                        if l0:
                            for s in range(4):
                                for kc in range(KC):
                                    idx = s * 8 + kc
                                    nc.tensor.matmul(
                                        ps[:, s, :, :], wrech[:, idx, :],
                                        h0buf[:, kc, :, :], start=(kc == 0),
                                        stop=False, skip_group_check=True)
                                    nc.tensor.matmul(
                                        ps[:, s, 0, :], wrecl[:, idx, :],
                                        h0buf[:, kc, 0, :], start=False,
                                        stop=(kc == KC - 1),
                                        skip_group_check=True)
                        if l1:
                            for s in range(4):
                                for kc in range(16):
                                    idx = 32 + s * 16 + kc
                                    src = h0buf if kc < 8 else h1buf
                                    kk = kc % 8
                                    nc.tensor.matmul(
                                        ps[:, 4 + s, :, :], wrech[:, idx, :],
                                        src[:, kk, :, :], start=(kc == 0),
                                        stop=False, skip_group_check=True)
                                    nc.tensor.matmul(
                                        ps[:, 4 + s, 0, :], wrecl[:, idx, :],
                                        src[:, kk, 0, :], start=False,
                                        stop=(kc == 15), skip_group_check=True)

                        # fold the wh*hl half into the main half, add x-term
                        fold = rsb.tile([128, 8, 64], F32, tag="tfold")
                        nc.any.tensor_copy(fold[:, lo_s:hi_s, :],
                                           ps[:, lo_s:hi_s, 1, :])
                        gsum = rsb.tile([128, 8, 64], F32, tag="tgsum")
                        nc.vector.tensor_tensor(gsum[:, lo_s:hi_s, :],
                                                ps[:, lo_s:hi_s, 0, :],
                                                fold[:, lo_s:hi_s, :], OP.add)
                        if l0:
                            g0xb = rpf.tile([128, 4, 64], F32, tag="g0xb")
                            nc.sync.dma_start(
                                g0xb[:], g0x_d[:, :, t * 64:(t + 1) * 64]
                                .rearrange("g p b -> p g b"))
                            nc.vector.tensor_tensor(gsum[:, 0:4, :],
                                                    gsum[:, 0:4, :], g0xb[:],
                                                    OP.add)
                        # gates (slices i,f,o,g): sigmoid via tanh(x/2)
                        sig = rsb.tile([128, 2, 3, 64], F32, tag="sig")
                        gth = rsb.tile([128, 2, 64], F32, tag="gth")
                        if l0:
                            nc.scalar.activation(sig[:, 0, :, :], gsum[:, 0:3, :],
                                                 ACT.Tanh, bias=0.0, scale=0.5)
                            nc.scalar.activation(gth[:, 0, :], gsum[:, 3, :],
                                                 ACT.Tanh)
                        if l1:
                            nc.scalar.activation(sig[:, 1, :, :], gsum[:, 4:7, :],
                                                 ACT.Tanh, bias=0.0, scale=0.5)
                            nc.scalar.activation(gth[:, 1, :], gsum[:, 7, :],
                                                 ACT.Tanh)
                        nc.vector.tensor_scalar(sig[:, lyr, :, :],
                                                sig[:, lyr, :, :],
                                                0.5, 0.5, OP.mult, OP.add)
                        t1 = rsb.tile([128, 2, 64], F32, tag="t1")
                        nc.vector.tensor_tensor(t1[:, lyr, :], sig[:, lyr, 0, :],
                                                gth[:, lyr, :], OP.mult)
                        t2 = rsb.tile([128, 2, 64], F32, tag="t2")
                        nc.vector.tensor_tensor(t2[:, lyr, :], sig[:, lyr, 1, :],
                                                c01[:, lyr, :], OP.mult)
                        nc.vector.tensor_tensor(c01[:, lyr, :], t1[:, lyr, :],
                                                t2[:, lyr, :], OP.add)
                        ct = rsb.tile([128, 2, 64], F32, tag="ct")
                        nc.scalar.activation(ct[:, lyr, :], c01[:, lyr, :],
                                             ACT.Tanh)
                        hloc = rsb.tile([128, 2, 64], F32, tag="hloc")
                        nc.vector.tensor_tensor(hloc[:, lyr, :],
                                                sig[:, lyr, 2, :],
                                                ct[:, lyr, :], OP.mult)
                        if t == 0:
                            nc.vector.tensor_copy(hloc[:, 1, :], h1chunk0[:])
                        if t == T:
                            nc.vector.tensor_copy(hloc[:, 0, :], hloc[:, 1, :])

                        # exchange: sender-side bf16 split halves the AG
                        # payload and lets receivers consume via pure DMA
                        hsp = rsb.tile([128, 2, 2, 64], BF16, tag="hsp")
                        nc.vector.tensor_copy(hsp[:, :, 0, :], hloc[:])
                        nc.vector.tensor_tensor(hsp[:, :, 1, :], hloc[:],
                                                hsp[:, :, 0, :], OP.subtract)
                        contrib = dram.tile([128, 256], BF16, tag="contrib")
                        gathered = dram.tile([1024, 256], BF16, tag="gathered")
                        nc.sync.dma_start(contrib[:],
                                          hsp[:].rearrange("p l s b -> p (l s b)"))
                        nc.gpsimd.collective_compute(
                            "AllGather", OP.bypass,
                            replica_groups=[list(range(NCORES))],
                            ins=[contrib.opt()], outs=[gathered.opt()])
                        gat_r = gathered.rearrange("(k p) c -> p k c", p=128)
                        if t < T:
                            nc.sync.dma_start(h0buf[:], gat_r[:, :, 0:128])
                        s_h1 = t - 1  # step of the gathered h1
                        if s_h1 >= 0:
                            cp = s_h1 * 64
                            nc.sync.dma_start(h1buf[:], gat_r[:, :, 128:256])
                            nc.sync.dma_start(h1Th[:, :, cp:cp + 64],
                                              gat_r[:, :, 128:192])
                            nc.sync.dma_start(h1lrot[:, :, s_h1 % 2, :],
                                              gat_r[:, :, 192:256])
                        # transpose finished 128-row tile (steps 2tau, 2tau+1)
                        if s_h1 >= 1 and s_h1 % 2 == 1:
                            tau = s_h1 // 2
                            sth = rsb.tile([128, H], BF16, tag="sth")
                            stl = rsb.tile([128, H], BF16, tag="stl")
                            for kc in range(KC):
                                nc.scalar.dma_start(
                                    sth[:, kc * 128:(kc + 1) * 128],
                                    h1Th[:, kc, 2 * tau * 64:2 * tau * 64 + 128],
                                    transpose=True)
                                nc.scalar.dma_start(
                                    stl[:, kc * 128:(kc + 1) * 128],
                                    h1lrot[:, kc, :, :], transpose=True)
                            nc.gpsimd.dma_start(
                                h1rh_d[tau * 128:(tau + 1) * 128, :], sth[:])
                            nc.gpsimd.dma_start(
                                h1rl_d[tau * 128:(tau + 1) * 128, :], stl[:])
                            do_projection(tau)

                # ============ Phase 4: stats exchange + combine ============
                NQ = 2 * NT + 2 * NT * 8  # 432
                with tc.tile_pool(name="cmb_sb", bufs=1) as csb:
                    cont = csb.tile([128, NQ], F32, tag="scont")
                    nc.vector.tensor_copy(cont[:, 0:NT],
                                          top8v[:, :, 0:1].rearrange("p t o -> p (t o)"))
                    nc.vector.tensor_copy(cont[:, NT:2 * NT], sumexp[:])
                    nc.vector.tensor_copy(
                        cont[:, 2 * NT:2 * NT + 192],
                        top8v[:].rearrange("p t k -> p (t k)"))
                    gidxf = csb.tile([128, NT, 8], F32, tag="gidxf")
                    nc.vector.tensor_copy(gidxf[:], top8i[:])
                    nc.vector.tensor_scalar(gidxf[:], gidxf[:], vbase[:], None,
                                            OP.add)
                    nc.vector.tensor_copy(
                        cont[:, 2 * NT + 192:NQ],
                        gidxf[:].rearrange("p t k -> p (t k)"))
                    scont_d = dram.tile([128, NQ], F32, tag="scont_d")
                    sgath_d = dram.tile([128 * NCORES, NQ], F32, tag="sgath_d")
                    nc.sync.dma_start(scont_d[:], cont[:])
                    nc.gpsimd.collective_compute(
                        "AllGather", OP.bypass,
                        replica_groups=[list(range(NCORES))],
                        ins=[scont_d.opt()], outs=[sgath_d.opt()])
                    gst = csb.tile([128, NCORES, NQ], F32, tag="gst")
                    nc.sync.dma_start(
                        gst[:], sgath_d.rearrange("(c p) q -> p c q", p=128))

                    # global logsumexp per row
                    maxv_v = gst[:, :, 0:NT].rearrange("p c t -> p t c")
                    sume_v = gst[:, :, NT:2 * NT].rearrange("p c t -> p t c")
                    gm = csb.tile([128, NT], F32, tag="gm")
                    nc.vector.tensor_reduce(gm[:], maxv_v, axis=AX.X, op=OP.max)
                    dsc = csb.tile([128, NT, NCORES], F32, tag="dsc")
                    nc.vector.tensor_tensor(
                        dsc[:], maxv_v,
                        gm[:].unsqueeze(2).broadcast_to((128, NT, NCORES)),
                        OP.subtract)
                    nc.scalar.activation(dsc[:], dsc[:], ACT.Exp)
                    nc.vector.tensor_tensor(dsc[:], dsc[:], sume_v, OP.mult)
                    S = csb.tile([128, NT], F32, tag="S")
                    nc.vector.tensor_reduce(S[:], dsc[:], axis=AX.X, op=OP.add)
                    nc.scalar.activation(S[:], S[:], ACT.Ln)
                    nc.vector.tensor_tensor(lse[:], S[:], gm[:], OP.add)

                    # global top-3 (value order) with first-occurrence index
                    cand_v = gst[:, :, 2 * NT:2 * NT + 192].rearrange(
                        "p c (t k) -> p t c k", t=NT)
                    gidx_v = gst[:, :, 2 * NT + 192:NQ].rearrange(
                        "p c (t k) -> p t c k", t=NT)
                    work = csb.tile([128, NT, NCORES, 8], F32, tag="work")
                    nc.vector.tensor_copy(work[:], cand_v)
                    scr1 = csb.tile([128, NT, NCORES, 8], F32, tag="scr1")
                    scr2 = csb.tile([128, NT, NCORES, 8], F32, tag="scr2")
                    isel = csb.tile([128, NT, NCORES, 8], F32, tag="isel")
                    for r in range(3):
                        vr = csb.tile([128, NT], F32, tag="vr")
                        nc.vector.tensor_reduce(vr[:], work[:], axis=AX.XY,
                                                op=OP.max)
                        nc.vector.tensor_tensor(
                            scr1[:], work[:],
                            vr[:].unsqueeze(2).unsqueeze(3).broadcast_to(
                                (128, NT, NCORES, 8)), OP.is_equal)
                        nc.vector.tensor_scalar(scr2[:], scr1[:], -1e9, 1e9,
                                                OP.mult, OP.add)
                        nc.vector.tensor_tensor(isel[:], gidx_v, scr1[:],
                                                OP.mult)
                        nc.vector.tensor_tensor(isel[:], isel[:], scr2[:],
                                                OP.add)
                        nc.vector.tensor_reduce(candi[:, r, :], isel[:],
                                                axis=AX.XY, op=OP.min)
                        if r < 2:
                            nc.vector.tensor_scalar(scr1[:], scr1[:], 2e30,
                                                    None, OP.mult)
                            nc.vector.tensor_tensor(work[:], work[:], scr1[:],
                                                    OP.subtract)

            # ============ Phase 5: rescore + loss + tokens ============
            with (
                tc.tile_pool(name="rsc_sb", bufs=2) as ssb,
                tc.tile_pool(name="rsc_st", bufs=1) as sst,
                tc.tile_pool(name="rsc_ps", bufs=1, space="PSUM") as sps,
            ):
                # select this core's 3 row-tiles from the [*, NT] stats
                def msel(src_nt, name):
                    """src [128, NT] -> [128, TPC] via per-core one-hot mask."""
                    scr = ssb.tile([128, TPC, NT], F32, tag="mscr")
                    nc.vector.tensor_tensor(
                        scr[:], src_nt.unsqueeze(1).broadcast_to(
                            (128, TPC, NT)), tmask[:], OP.mult)
                    out = sst.tile([128, TPC], F32, tag=name)
                    nc.vector.tensor_reduce(out[:], scr[:], axis=AX.X, op=OP.add)
                    return out

                tgt_s = msel(tgtf[:], "tgt_s")
                lw_s = msel(lossw[:], "lw_s")
                lse_s = msel(lse[:], "lse_s")
                cand_s = [msel(candi[:, r, :], f"cand_s{r}") for r in range(3)]

                rowu = sst.tile([128, TPC], U32, tag="rowu")
                nc.vector.tensor_copy(rowu[:], rowsel[:])
                scat = sst.tile([128, TPC], F32, tag="scat")
                nc.vector.tensor_scalar(scat[:], rowsel[:], float(B), None,
                                        OP.add)
                scatu = sst.tile([128, TPC], U32, tag="scatu")
                nc.vector.tensor_copy(scatu[:], scat[:])

                dots = sst.tile([128, TPC, 4], F32, tag="dots")
                for i in range(TPC):
                    h1h_t = ssb.tile([128, H], BF16, tag="h1h_t")
                    nc.gpsimd.indirect_dma_start(
                        out=h1h_t[:], out_offset=None, in_=h1rh_d,
                        in_offset=bass.IndirectOffsetOnAxis(
                            ap=rowu[:, i:i + 1], axis=0))
                    h1l_t = ssb.tile([128, H], BF16, tag="h1l_t")
                    nc.gpsimd.indirect_dma_start(
                        out=h1l_t[:], out_offset=None, in_=h1rl_d,
                        in_offset=bass.IndirectOffsetOnAxis(
                            ap=rowu[:, i:i + 1], axis=0))
                    h1row = ssb.tile([128, H], F32, tag="h1row")
                    nc.vector.tensor_copy(h1row[:], h1h_t[:])
                    nc.vector.tensor_tensor(h1row[:], h1row[:], h1l_t[:],
                                            OP.add)
                    for ci in range(4):
                        src = cand_s[ci][:, i:i + 1] if ci < 3 \
                            else tgt_s[:, i:i + 1]
                        cu = ssb.tile([128, 1], U32, tag="cu")
                        nc.vector.tensor_copy(cu[:], src)
                        wrow = ssb.tile([128, H], F32, tag="wrow")
                        nc.gpsimd.indirect_dma_start(
                            out=wrow[:], out_offset=None, in_=wout_in,
                            in_offset=bass.IndirectOffsetOnAxis(
                                ap=cu[:, 0:1], axis=0))
                        prod = ssb.tile([128, H], F32, tag="prod")
                        nc.vector.tensor_tensor(prod[:], h1row[:], wrow[:],
                                                OP.mult)
                        nc.vector.tensor_reduce(dots[:, i, ci:ci + 1], prod[:],
                                                axis=AX.X, op=OP.add)

                # winner among the 3 rescored candidates (exact values)
                best = sst.tile([128, TPC], F32, tag="best")
                nc.vector.tensor_copy(best[:], dots[:, :, 0])
                bidx = sst.tile([128, TPC], F32, tag="bidx")
                nc.vector.tensor_copy(bidx[:], cand_s[0][:])
                for ci in (1, 2):
                    m = sst.tile([128, TPC], U32, tag="m")
                    nc.vector.tensor_tensor(m[:], dots[:, :, ci], best[:],
                                            OP.is_gt)
                    nc.vector.copy_predicated(best[:], m[:], dots[:, :, ci])
                    nc.vector.copy_predicated(bidx[:], m[:], cand_s[ci][:])
                toki = sst.tile([128, TPC], I32, tag="toki")
                nc.vector.tensor_copy(toki[:], bidx[:])
                # scatter tokens into result[1+t, b] (flat offset r + 64)
                res_flat = result_out.rearrange("t b -> (t b)").unsqueeze(1)
                for i in range(TPC):
                    nc.gpsimd.indirect_dma_start(
                        out=res_flat, out_offset=bass.IndirectOffsetOnAxis(
                            ap=scatu[:, i:i + 1], axis=0),
                        in_=toki[:, i:i + 1], in_offset=None,
                        bounds_check=TB - 1, oob_is_err=False)
                ones_row = sst.tile([64, 1], I32, tag="ones_row")
                nc.vector.memset(ones_row[:], 1)
                nc.sync.dma_start(res_flat[0:B], ones_row[:])

                # loss = sum over rows of (lse - exact_tgt_dot) * lossw
                ce = sst.tile([128, TPC], F32, tag="ce")
                nc.vector.tensor_tensor(ce[:], lse_s[:], dots[:, :, 3],
                                        OP.subtract)
                nc.vector.tensor_tensor(ce[:], ce[:], lw_s[:], OP.mult)
                part = sst.tile([128, 1], F32, tag="part")
                nc.vector.tensor_reduce(part[:], ce[:], axis=AX.X, op=OP.add)
                ar_in = dram.tile([128, 1], F32, tag="ar_in")
                ar_out = dram.tile([128, 1], F32, tag="ar_out")
                nc.sync.dma_start(ar_in[:], part[:])
                nc.gpsimd.collective_compute(
                    "AllReduce", OP.add,
                    replica_groups=[list(range(NCORES))],
                    ins=[ar_in.opt()], outs=[ar_out.opt()])
                summed = sst.tile([128, 1], F32, tag="summed")
                nc.sync.dma_start(summed[:], ar_out[:])
                onesf = sst.tile([128, 1], F32, tag="onesf")
                nc.vector.memset(onesf[:], 1.0)
                lps = sps.tile([1, 1], F32, tag="lps")
                nc.tensor.matmul(lps[:], onesf[:], summed[:], start=True,
                                 stop=True)
                lsb = sst.tile([1, 1], F32, tag="lsb")
                nc.vector.tensor_copy(lsb[:], lps[:])
                nc.sync.dma_start(loss_out, lsb[:])

    nc.compile()
    _CACHE["nc"] = nc
    return nc


def _split(x):
    h = np.asarray(x, np.float32).astype(BF)
    l = (np.asarray(x, np.float32) - h.astype(np.float32)).astype(BF)
    return h, l


def prep_inputs(output_tensor, hidden_state, cell_state, embedding,
                w_ih, w_hh, W_out, **_unused):
    """Host-side sharding/layout prep. Returns per-core input maps."""
    tok = np.asarray(output_tensor)
    emb = np.asarray(embedding, np.float32).copy()
    emb[0] = 0.0
    w_ih = np.asarray(w_ih, np.float32)
    w_hh = np.asarray(w_hh, np.float32)
    W_out = np.ascontiguousarray(np.asarray(W_out, np.float32))
    h_st = np.asarray(hidden_state, np.float32)
    c_st = np.asarray(cell_state, np.float32)

    tok_in = np.concatenate([tok[:TS].T.reshape(-1),
                             np.zeros(B, np.int64)])  # [TB] t-major, b fast
    # careful: rows r = t*64+b -> in token order tok[t, b]
    tok_in = np.zeros(TB, np.int64)
    tgt = np.zeros(TB, np.int64)
    tgrid = np.asarray(tok)
    for t in range(TS):
        tok_in[t * B:(t + 1) * B] = tgrid[t]
        tgt[t * B:(t + 1) * B] = tgrid[t + 1]
    # step 47 (dummy): tokens 0, tgt 0 (masked)

    X = emb[tok_in]                       # [TB, 1024]
    XT = np.ascontiguousarray(X.T)        # [1024, TB]
    xh, xl = _split(XT)
    xTh = xh.reshape(KC, 128, TB)
    xTl = xl.reshape(KC, 128, TB)

    # per-row loss weights: mask/denom ; tgtf
    mask = (tgt != 0).astype(np.float32)
    mask[TS * B:] = 0.0
    lw = np.zeros(TB, np.float32)
    for t in range(TS):
        s = mask[t * B:(t + 1) * B].sum()
        lw[t * B:(t + 1) * B] = mask[t * B:(t + 1) * B] / max(s, 1.0)
    # SBUF layout [128 partitions, NT]: row r = tau*128 + p
    lw_pt = lw.reshape(NT, 128).T.copy()           # [128, NT]
    tgt_pt = tgt.astype(np.float32).reshape(NT, 128).T.copy()

    # gate-block order in this kernel: i, f, o, g  (PyTorch rows: i,f,g,o)
    GT_ROWS = [0, H, 3 * H, 2 * H]  # start row of i, f, o, g in [4H]

    in_maps = []
    for j in range(NCORES):
        hj = slice(128 * j, 128 * (j + 1))

        def rec_tiles(mats):
            """mats: list of (matrix [4H or G, K], kc-range) stacked tiles."""
            tiles_h = []
            tiles_l = []
            for s, g0 in enumerate(GT_ROWS):
                for (mat, kcn) in mats:
                    rows = mat[g0 + 128 * j: g0 + 128 * (j + 1)]  # [128, K]
                    for kc in range(kcn):
                        blk = rows[:, kc * 128:(kc + 1) * 128].T  # [128k,128m]
                        bh, bl = _split(blk)
                        tiles_h.append(bh)
                        tiles_l.append(bl)
            return np.stack(tiles_h), np.stack(tiles_l)

        wih0h_t, wih0l_t = rec_tiles([(w_ih[0], KC)])
        wrec_l0h, wrec_l0l = rec_tiles([(w_hh[0], KC)])
        wrec_l1h, wrec_l1l = rec_tiles([(w_ih[1], KC), (w_hh[1], KC)])
        wrech_t = np.concatenate([wrec_l0h, wrec_l1h])
        wrecl_t = np.concatenate([wrec_l0l, wrec_l1l])

        Wj = np.zeros((VPC, H), np.float32)
        Wj[:VREAL] = W_out[VREAL * j:VREAL * (j + 1)]
        wpj_h = Wj.T.astype(BF)                      # [1024, 4096] bf16
        wproj_t = wpj_h.reshape(KC, 128, VPC)

        h0T = np.ascontiguousarray(h_st[0].T)        # [1024, 64]
        h1T = np.ascontiguousarray(h_st[1].T)
        def init_pack(hT):
            hh, hl = _split(hT)
            out = np.zeros((128, KC, 2, B), BF)
            for kc in range(KC):
                out[:, kc, 0, :] = hh[kc * 128:(kc + 1) * 128]
                out[:, kc, 1, :] = hl[kc * 128:(kc + 1) * 128]
            return out
        cinit = np.zeros((128, 2, B), np.float32)
        cinit[:, 0, :] = c_st[0].T[hj]
        cinit[:, 1, :] = c_st[1].T[hj]

        rowsel = np.zeros((128, TPC), np.float32)
        tmask_np = np.zeros((128, TPC, NT), np.float32)
        for i in range(TPC):
            tau = TPC * j + i
            rowsel[:, i] = 128 * tau + np.arange(128)
            tmask_np[:, i, tau] = 1.0

        in_maps.append({
            "xTh_in": xTh, "xTl_in": xTl,
            "wih0h_in": wih0h_t, "wih0l_in": wih0l_t,
            "wrech_in": wrech_t, "wrecl_in": wrecl_t,
            "wproj_in": wproj_t, "wout_in": W_out,
            "h0init_in": init_pack(h0T), "h1init_in": init_pack(h1T),
            "h1chunk_in": np.ascontiguousarray(h1T[hj]),
            "cinit_in": cinit,
            "vbase_in": np.full((128, 1), VREAL * j, np.float32),
            "tgtf_in": tgt_pt, "lossw_in": lw_pt,
            "rowsel_in": rowsel, "tmask_in": tmask_np,
        })
    return in_maps


def assemble(results, out_dtype):
    """Per-core results -> (loss [1] f32, result [48,64] out_dtype)."""
    loss = np.asarray(results[0]["loss_out"]).reshape(1).astype(np.float32)
    result = np.ones((TS + 1, B), np.int64)
    for j in range(NCORES):
        res_j = np.asarray(results[j]["result_out"])  # [48, 64]
        for i in range(TPC):
            tau = TPC * j + i
            for half in range(2):
                s = 2 * tau + half          # step index
                if s >= TS:
                    continue
                result[1 + s] = res_j[1 + s]
    return loss, result.astype(out_dtype)


def kernel(**inputs):
    nc = build_nc()
    in_maps = prep_inputs(**inputs)
    r = bass_utils.run_bass_kernel_spmd(
        nc, in_maps, core_ids=list(range(NCORES)))
    out_dtype = np.asarray(inputs["output_tensor"]).dtype
    return assemble(r.results, out_dtype)


if __name__ == "__main__":
    build_nc()
    print("built ok")
